# revision 22
# baseline (speedup 1.0000x reference)
"""Multi-head self-attention TRN2 Bass kernel (fused pipeline).

Problem: B=4, N=2048, C=1024, H=16 heads, D=64. 8 NeuronCores.
Sharding: core c handles batch b=c//2, head-group g=c%2 (8 heads each).
Data parallel on B, tensor parallel on heads; proj is row-parallel with the
partial sums combined on the host.

Everything on-device is computed in "transposed land" so no transposes are
ever needed:
  - host feeds x^T; qkv biases are added by the DVE during the PSUM
    evacuation copies (free: tensor_tensor add instead of tensor_copy);
    all operands bf16
  - q^T,k^T computed feature-major [feat, tok]; v token-major [tok, feat]
  - attention runs one head at a time over nq blocks of 1024:
    scores^T tile = matmul(lhsT=k^T chunk [64,128], rhs=q^T block), K=64
  - exp on ScalarE (softmax max-subtraction skipped: scores are ~N(0,0.33),
    bounded well inside fp32 exp range), FD=1024 per instruction
  - AV^T = matmul(lhsT=v_aug [nk,66] with a ones column, rhs=P^T) so the
    softmax denominator Z accumulates in row 64 of the same PSUM tile
  - 1/Z broadcast across partitions on the (otherwise idle) GPSIMD engine
    via partition_broadcast; normalize via DVE multiply
  - proj = matmul(lhsT=Wp^T, rhs=o_norm^T) -> out^T partial, fp32 to HBM

All phases are emitted fused so the Tile scheduler software-pipelines them:
qkv projection and output projection act as TensorE filler work while the
ScalarE exp stream (the per-core softmax floor, ~255us) drains.
"""

import numpy as np
import ml_dtypes
from contextlib import ExitStack

N_CORES = 8
B, N, C = 4, 2048, 1024
H, D = 16, 64
HL = H // 2          # heads per core (8)
CL = HL * D          # local features per head-group (512)
KC = 8               # contraction chunks (biases folded via DVE, not ones-row)
CA = KC * 128        # contraction size (1024)
TB = 4               # token blocks of 512 for qkv/proj
NKC = 16             # nk chunks of 128
VW = 66              # v lane width: 64 dims + ones col + zero pad (even for DVE 2x)
BF = ml_dtypes.bfloat16

_CACHE = {}


def _build(loop_n=1):
    import concourse.tile as tile
    from concourse import bacc, mybir

    bf = mybir.dt.bfloat16
    f32 = mybir.dt.float32
    AF = mybir.ActivationFunctionType

    nc = bacc.Bacc("TRN2", target_bir_lowering=False, debug=False,
                   num_devices=N_CORES)
    xT = nc.dram_tensor("xT", [CA, N], bf, kind="ExternalInput").ap()
    wqk = nc.dram_tensor("wqk", [CA, 2 * CL], bf, kind="ExternalInput").ap()
    wv = nc.dram_tensor("wv", [CA, CL], bf, kind="ExternalInput").ap()
    wp = nc.dram_tensor("wp", [CL, C], bf, kind="ExternalInput").ap()
    wb = nc.dram_tensor("wb", [128, 8], bf, kind="ExternalInput").ap()
    vb = nc.dram_tensor("vb", [1, CL], bf, kind="ExternalInput").ap()
    outT = nc.dram_tensor("outT", [C, N], f32, kind="ExternalOutput").ap()

    xT_r = xT.rearrange("(k p) n -> k p n", p=128)
    wqk_r = wqk.rearrange("(k p) n -> k p n", p=128)
    wv_r = wv.rearrange("(k p) n -> k p n", p=128)
    wp_r = wp.rearrange("(k p) n -> k p n", p=128)

    with tile.TileContext(nc) as tc, ExitStack() as ctx:
        const = ctx.enter_context(tc.tile_pool(name="const", bufs=1))
        x_bufs = [const.tile([128, KC, N], bf, name="x%d" % i)
                  for i in range(2 if loop_n > 1 else 1)]
        wqk_sb = const.tile([128, KC, 2 * CL], bf)
        wv_sb = const.tile([128, KC, CL], bf)
        wp_sb = const.tile([128, 4, C], bf)
        qk_sb = const.tile([128, 8, N], bf)        # [feat%128, feat_tile, tok]
        v_sb = const.tile([128, NKC, HL * VW], bf)  # v + ones col + pad per head
        o_sb = const.tile([128, 4, N], bf)         # o_norm^T [cloc%128, chunk, tok]

        # PSUM: s 2x2 banks + av 1x2 banks + mm 2x1 banks = 8 banks exactly
        sps = ctx.enter_context(tc.tile_pool(name="sps", bufs=2, space="PSUM"))
        avps = ctx.enter_context(tc.tile_pool(name="avps", bufs=1, space="PSUM"))
        mmps = ctx.enter_context(tc.tile_pool(name="mmps", bufs=2, space="PSUM"))
        pp = ctx.enter_context(tc.tile_pool(name="pp", bufs=6))
        normp = ctx.enter_context(tc.tile_pool(name="normp", bufs=1))
        onp = ctx.enter_context(tc.tile_pool(name="onp", bufs=2))
        osp = ctx.enter_context(tc.tile_pool(name="osp", bufs=4))

        # weights resident across iterations
        for k in range(KC):
            nc.scalar.dma_start(wqk_sb[:, k, :], wqk_r[k])
            nc.scalar.dma_start(wv_sb[:, k, :], wv_r[k])
        for k in range(4):
            nc.scalar.dma_start(wp_sb[:, k, :], wp_r[k])
        wb_sb = const.tile([128, 8], bf)
        vb_row = const.tile([1, CL], bf)
        vb_sb = const.tile([128, CL], bf)   # v bias replicated across tokens
        nc.scalar.dma_start(wb_sb[:], wb)
        nc.scalar.dma_start(vb_row[:], vb)
        nc.gpsimd.partition_broadcast(vb_sb[:], vb_row[:], channels=128)
        v_hl = v_sb.rearrange("p t (h e) -> p t h e", e=VW)
        nc.vector.memset(v_hl[:, :, :, 64:65], 1.0)
        nc.vector.memset(v_hl[:, :, :, 65:66], 0.0)

        def _qk_pair(x_sb, f, parts=(0, 1), tb_outer=False):
            # feature tiles f (q) and 4+f (k); weight chunk stays in flight
            # only per-MM (LDW hides under the N=512 stream). tb_outer
            # consumes x token-slices as the DMA delivers them (startup).
            order = ([(tb, sel) for tb in range(TB) for sel in parts]
                     if tb_outer else
                     [(tb, sel) for sel in parts for tb in range(TB)])
            for tb, sel in order:
                ft = f if sel == 0 else 4 + f
                ps = mmps.tile([128, 512], f32, tag="mm")
                for k in range(KC):
                    nc.tensor.matmul(
                        ps[:],
                        wqk_sb[:, k, ft * 128:(ft + 1) * 128],
                        x_sb[:, k, tb * 512:(tb + 1) * 512],
                        start=(k == 0), stop=(k == KC - 1),
                    )
                nc.vector.tensor_add(
                    qk_sb[:, ft, tb * 512:(tb + 1) * 512], ps[:],
                    wb_sb[:, ft:ft + 1].to_broadcast([128, 512]))

        def _v_tile(x_sb, tt):
            ps = mmps.tile([128, 512], f32, tag="mm")
            for k in range(KC):
                nc.tensor.matmul(
                    ps[:],
                    x_sb[:, k, tt * 128:(tt + 1) * 128],
                    wv_sb[:, k, :],
                    start=(k == 0), stop=(k == KC - 1),
                )
            v_out = v_sb[:, tt, :].rearrange("p (h e) -> p h e", e=VW)[:, :, 0:64]
            v_in = ps[:].rearrange("p (h e) -> p h e", e=64)
            nc.vector.tensor_add(v_out, v_in,
                                 vb_sb.rearrange("p (h e) -> p h e", e=64))

        def _attn_blk(h, blk, fill=None):
            hrow = 64 * (h % 2)
            f = h // 2
            nq0 = blk * 1024
            q_ap = qk_sb[hrow:hrow + 64, f, :]
            k_ap = qk_sb[hrow:hrow + 64, 4 + f, :]
            av = avps.tile([VW, 1024], f32, tag="av")
            for ck in range(NKC):
                s = sps.tile([128, 1024], f32, tag="s")
                for q in range(2):
                    nc.tensor.matmul(
                        s[:, q * 512:(q + 1) * 512],
                        k_ap[:, ck * 128:(ck + 1) * 128],
                        q_ap[:, nq0 + q * 512:nq0 + (q + 1) * 512],
                        start=True, stop=True)
                p = pp.tile([128, 1024], bf, tag="p")
                nc.scalar.activation(p[:], s[:], AF.Exp)
                if fill is not None:
                    fill(ck)
                for q in range(2):
                    nc.tensor.matmul(
                        av[:, q * 512:(q + 1) * 512],
                        v_sb[:, ck, h * VW:(h + 1) * VW],
                        p[:, q * 512:(q + 1) * 512],
                        start=(ck == 0), stop=(ck == NKC - 1))
            # normalize: o = av[0:64] * (1/Z), Z = av row 64.
            # Evacuation copies release the PSUM accumulator ~2.3us after
            # the last AV matmul; 1/Z via reciprocal_approx_fast (51 ULP,
            # fp32 SBUF->SBUF) instead of the ~8 cyc/elem iterative-divide
            # reciprocal(), which on a single-lane [1,1024] AP costs ~8.5us.
            oc = normp.tile([64, 1024], bf, tag="oc")
            with nc.allow_low_precision(reason="o in bf16 (o_sb is bf16)"):
                nc.vector.tensor_copy(oc[:], av[0:64, :])
            zc = normp.tile([1, 1024], f32, tag="zc")
            nc.vector.tensor_copy(zc[:], av[64:65, :])
            r32 = normp.tile([1, 1024], f32, tag="r32")
            nc.vector.reciprocal_approx_fast(r32[:], zc[:])
            r = normp.tile([1, 1024], bf, tag="r")
            with nc.allow_low_precision(reason="1/Z in bf16; validated e2e"):
                nc.vector.tensor_copy(r[:], r32[:])
            bc = normp.tile([64, 1024], bf, tag="bc")
            nc.gpsimd.partition_broadcast(bc[:], r[:], channels=64)
            if h % 2 == 0:
                nc.vector.tensor_mul(
                    o_sb[0:64, f, nq0:nq0 + 1024], oc[:], bc[:])
            else:
                on = onp.tile([64, 1024], bf, tag="on")
                nc.vector.tensor_mul(on[:], oc[:], bc[:])
                nc.scalar.dma_start(o_sb[64:128, f, nq0:nq0 + 1024], on[:])

        def _proj_blk(blk):
            # wp chunk stays stationary for the 2 matmuls of the tb pair
            for ct in range(8):
                pss = [mmps.tile([128, 512], f32, tag="mm", name="pj%d" % i)
                       for i in range(2)]
                for k in range(4):
                    for i in range(2):
                        tb = blk * 2 + i
                        nc.tensor.matmul(
                            pss[i][:],
                            wp_sb[:, k, ct * 128:(ct + 1) * 128],
                            o_sb[:, k, tb * 512:(tb + 1) * 512],
                            start=(k == 0), stop=(k == 3),
                        )
                for i in range(2):
                    tb = blk * 2 + i
                    ost = osp.tile([128, 512], f32, tag="o")
                    nc.vector.tensor_copy(ost[:], pss[i][:])
                    nc.scalar.dma_start(
                        outT[ct * 128:(ct + 1) * 128,
                             tb * 512:(tb + 1) * 512],
                        ost[:])

        def _dma_x(x_sb):
            # x streamed token-block-major on the sync queue (weights use the
            # scalar queue) so the first qkv matmuls start after ~1/4 of it
            for tb in range(TB):
                for k in range(KC):
                    nc.sync.dma_start(
                        x_sb[:, k, tb * 512:(tb + 1) * 512],
                        xT_r[k][:, tb * 512:(tb + 1) * 512])

        def _body(x_sb, x_next=None, head_done=False):
            # Cross-body software pipeline: when head_done, this body's x
            # was prefetched and its qk(0,4) already ran as filler in the
            # previous body's ACT-paced h5/h6 stretch.  x_next's DMA is
            # issued here so it streams during this body; x_next's qk(0,4)
            # is emitted as filler below (WAR-safe: it only overwrites
            # qk_sb ft0/ft4, which feed heads 0/1, long done by h5).
            if not head_done:
                _dma_x(x_sb)
                _qk_pair(x_sb, 0, tb_outer=True)
            if x_next is not None:
                _dma_x(x_next)
            for tt in range(4):
                _v_tile(x_sb, tt)
            # v tiles 4..15 interleave into head 0's first block (emitted
            # ahead of the AV matmuls that consume them)
            _attn_blk(0, 0,
                      fill=lambda ck: _v_tile(x_sb, ck + 4) if ck < 12 else None)
            _attn_blk(0, 1)
            _attn_blk(1, 0)
            _qk_pair(x_sb, 1, parts=(0,))
            _attn_blk(1, 1)
            _qk_pair(x_sb, 1, parts=(1,))
            _attn_blk(2, 0)
            _attn_blk(2, 1)
            _qk_pair(x_sb, 2, parts=(0,))
            _attn_blk(3, 0)
            _qk_pair(x_sb, 2, parts=(1,))
            _attn_blk(3, 1)
            _attn_blk(4, 0)
            _qk_pair(x_sb, 3, parts=(0,))
            _attn_blk(4, 1)
            _qk_pair(x_sb, 3, parts=(1,))
            _attn_blk(5, 0)
            _attn_blk(5, 1)
            _attn_blk(6, 0)
            _attn_blk(7, 0)
            _proj_blk(0)
            _attn_blk(7, 1)
            _attn_blk(6, 1)
            _proj_blk(1)
            if x_next is not None:
                # next body's qk(0,4) as pure tail filler: overlaps this
                # body's norm/proj/drain tail without displacing any of its
                # own score delivery to the exp stream
                _qk_pair(x_next, 0, tb_outer=True)

        if loop_n > 1:
            assert loop_n % 4 == 0, "loop_n must be a multiple of 4"
            with tc.For_i(0, loop_n // 4, 1,
                          hint_engines=(mybir.EngineType.PE,
                                        mybir.EngineType.Activation)):
                # 4 bodies alternating 2 x buffers; bodies 1-3 have their x
                # and qk(0,4) head prefetched by the previous body's section
                for rep in range(4):
                    _body(x_bufs[rep % 2],
                          x_next=x_bufs[(rep + 1) % 2] if rep < 3 else None,
                          head_done=(rep > 0))
        else:
            _body(x_bufs[0])

    nc.compile()
    return nc


def _prep_core_inputs(x, w_qkv, b_qkv, w_proj, core):
    b, g = core // 2, core % 2
    scale = np.float32(D) ** -0.5

    xT = np.ascontiguousarray(x[b].T).astype(BF)

    q_w = w_qkv[g * CL:(g + 1) * CL] * scale
    k_w = w_qkv[C + g * CL:C + (g + 1) * CL]
    v_w = w_qkv[2 * C + g * CL:2 * C + (g + 1) * CL]
    q_b = b_qkv[g * CL:(g + 1) * CL] * scale
    k_b = b_qkv[C + g * CL:C + (g + 1) * CL]
    v_b = b_qkv[2 * C + g * CL:2 * C + (g + 1) * CL]

    wqk_c = np.empty((CA, 2 * CL), dtype=BF)
    wqk_c[:, :CL] = q_w.T.astype(BF)
    wqk_c[:, CL:] = k_w.T.astype(BF)

    # per-feature-tile bias columns: ft 0-3 = q tiles, 4-7 = k tiles
    wb = np.empty((128, 8), dtype=BF)
    for f in range(4):
        wb[:, f] = q_b[f * 128:(f + 1) * 128].astype(BF)
        wb[:, 4 + f] = k_b[f * 128:(f + 1) * 128].astype(BF)

    wpT = np.ascontiguousarray(w_proj[:, g * CL:(g + 1) * CL].T).astype(BF)

    return {"xT": xT, "wqk": wqk_c, "wv": v_w.T.astype(BF),
            "wp": wpT, "wb": wb, "vb": v_b.astype(BF).reshape(1, CL)}


def kernel(x, w_qkv, b_qkv, w_proj, b_proj):
    from concourse.bass_utils import run_bass_kernel_spmd

    x = np.asarray(x, dtype=np.float32)
    w_qkv = np.asarray(w_qkv, dtype=np.float32)
    b_qkv = np.asarray(b_qkv, dtype=np.float32)
    w_proj = np.asarray(w_proj, dtype=np.float32)
    b_proj = np.asarray(b_proj, dtype=np.float32)

    if "nc" not in _CACHE:
        _CACHE["nc"] = _build()
    nc = _CACHE["nc"]

    in_maps = [_prep_core_inputs(x, w_qkv, b_qkv, w_proj, c)
               for c in range(N_CORES)]
    res = run_bass_kernel_spmd(nc, in_maps, core_ids=list(range(N_CORES)))
    _CACHE["last_results"] = res

    out = np.empty((B, N, C), dtype=np.float32)
    for b in range(B):
        acc = res.results[2 * b]["outT"] + res.results[2 * b + 1]["outT"]
        out[b] = acc.T + b_proj[None, :]
    return out


def _make_runner(nc, in_maps):
    """jit-wrap a compiled module for repeated dispatch on the 8 cores."""
    import jax
    from concourse import bass2jax, mybir
    from jax.sharding import Mesh, PartitionSpec, NamedSharding

    bass2jax.install_neuronx_cc_hook()
    part_name = (nc.partition_id_tensor.name
                 if nc.partition_id_tensor is not None else None)
    in_names, out_names, out_avals, zero_outs = [], [], [], []
    for alloc in nc.m.functions[0].allocations:
        if not isinstance(alloc, bass2jax.mybir.MemoryLocationSet):
            continue
        name = alloc.memorylocations[0].name
        if alloc.kind == "ExternalInput":
            if name != part_name:
                in_names.append(name)
        elif alloc.kind == "ExternalOutput":
            out_names.append(name)
            shape = tuple(alloc.tensor_shape)
            dtype = mybir.dt.np(alloc.dtype)
            out_avals.append(jax.core.ShapedArray(shape, dtype))
            zero_outs.append(np.zeros(shape, dtype))
    n_params = len(in_names)
    n_outs = len(out_avals)
    all_names = in_names + out_names
    if part_name is not None:
        all_names = all_names + [part_name]

    def _bodyfn(*args):
        operands = list(args)
        if part_name is not None:
            operands.append(bass2jax.partition_id_tensor())
        outs = bass2jax._bass_exec_p.bind(
            *operands,
            out_avals=tuple(out_avals),
            in_names=tuple(all_names),
            out_names=tuple(out_names),
            lowering_input_output_aliases=(),
            sim_require_finite=True,
            sim_require_nnan=True,
            nc=nc,
        )
        return tuple(outs)

    devices = jax.devices()[:N_CORES]
    mesh = Mesh(np.asarray(devices), ("core",))
    spec = PartitionSpec("core")
    sharded = jax.jit(
        bass2jax.shard_map(_bodyfn, mesh=mesh,
                           in_specs=(spec,) * (n_params + n_outs),
                           out_specs=(spec,) * n_outs, check_rep=False),
        keep_unused=True)

    concat_in = [
        np.concatenate([np.asarray(in_maps[c][name]) for c in range(N_CORES)],
                       axis=0)
        for name in in_names
    ]
    sh = NamedSharding(mesh, spec)
    dev_in = [jax.device_put(a, sh) for a in concat_in]
    dev_zeros = [jax.device_put(
        np.zeros((N_CORES * z.shape[0], *z.shape[1:]), z.dtype), sh)
        for z in zero_outs]

    def run():
        outs = sharded(*dev_in, *dev_zeros)
        jax.block_until_ready(outs)
        return outs

    return run


def benchmark(x, w_qkv, b_qkv, w_proj, b_proj, iters=20, reps=4):
    """HW execution time of one kernel body.

    Measured as the slope between two hardware For-loop NEFFs running the
    identical body `iters` and `iters+100` times on-device: per-body ns =
    (T_long - T_short) / 100.  The differencing cancels everything that is
    not device execution of the body (host dispatch RPC, NEFF entry/exit,
    input/output transfer), which otherwise dominates wall-clock here
    (~60ms/dispatch through the axon tunnel vs ~0.5ms of device time).
    Both points are best-of-`reps` dispatches.

    Test-harness helper only (not used by kernel()).
    """
    import time

    x = np.asarray(x, dtype=np.float32)
    in_maps = [_prep_core_inputs(x, np.asarray(w_qkv, np.float32),
                                 np.asarray(b_qkv, np.float32),
                                 np.asarray(w_proj, np.float32), c)
               for c in range(N_CORES)]

    n1 = max(4, ((iters + 3) // 4) * 4)   # loop NEFF needs a multiple of 4
    n2 = n1 + 100
    runners = {}
    for loop_n in (n1, n2):
        key = ("loop", loop_n)
        if key not in _CACHE:
            _CACHE[key] = _build(loop_n=loop_n)
        runners[loop_n] = _make_runner(_CACHE[key], in_maps)
        runners[loop_n]()  # warmup: compiles + loads NEFF
    # Alternate the two points so transient host/device slowness (e.g.
    # post-reset warmup) cannot land entirely on one point and skew the
    # slope; best-of per point over the interleaved reps.
    best = {n1: None, n2: None}
    for _ in range(max(reps, 5)):
        for loop_n in (n1, n2):
            t0 = time.perf_counter()
            runners[loop_n]()
            t1 = time.perf_counter()
            dt = t1 - t0
            if best[loop_n] is None or dt < best[loop_n]:
                best[loop_n] = dt
    return (best[n2] - best[n1]) / (n2 - n1) * 1e9


# revision 23
# speedup vs baseline: 1.8890x; 1.8890x over previous
"""Multi-head self-attention TRN2 Bass kernel (fused pipeline).

Problem: B=4, N=2048, C=1024, H=16 heads, D=64. 8 NeuronCores.
Sharding: core c handles batch b=c//2, head-group g=c%2 (8 heads each).
Data parallel on B, tensor parallel on heads; proj is row-parallel with the
partial sums combined on the host.

Everything on-device is computed in "transposed land" so no transposes are
ever needed:
  - host feeds x^T; qkv biases are added by the DVE during the PSUM
    evacuation copies (free: tensor_tensor add instead of tensor_copy);
    all operands bf16
  - q^T,k^T computed feature-major [feat, tok]; v token-major [tok, feat]
  - attention runs one head at a time over nq blocks of 1024:
    scores^T tile = matmul(lhsT=k^T chunk [64,128], rhs=q^T block), K=64
  - exp on ScalarE (softmax max-subtraction skipped: scores are ~N(0,0.33),
    bounded well inside fp32 exp range), FD=1024 per instruction
  - AV^T = matmul(lhsT=v_aug [nk,66] with a ones column, rhs=P^T) so the
    softmax denominator Z accumulates in row 64 of the same PSUM tile
  - 1/Z broadcast across partitions on the (otherwise idle) GPSIMD engine
    via partition_broadcast; normalize via DVE multiply
  - proj = matmul(lhsT=Wp^T, rhs=o_norm^T) -> out^T partial, fp32 to HBM

All phases are emitted fused so the Tile scheduler software-pipelines them:
qkv projection and output projection act as TensorE filler work while the
ScalarE exp stream (the per-core softmax floor, ~255us) drains.
"""

import numpy as np
import ml_dtypes
from contextlib import ExitStack

N_CORES = 8
B, N, C = 4, 2048, 1024
H, D = 16, 64
HL = H // 2          # heads per core (8)
CL = HL * D          # local features per head-group (512)
KC = 8               # contraction chunks (biases folded via DVE, not ones-row)
CA = KC * 128        # contraction size (1024)
TB = 4               # token blocks of 512 for qkv/proj
NKC = 16             # nk chunks of 128
VW = 66              # v lane width: 64 dims + ones col + zero pad (even for DVE 2x)
BF = ml_dtypes.bfloat16

_CACHE = {}


def _build(loop_n=1):
    import concourse.tile as tile
    from concourse import bacc, mybir

    bf = mybir.dt.bfloat16
    f32 = mybir.dt.float32
    AF = mybir.ActivationFunctionType

    nc = bacc.Bacc("TRN2", target_bir_lowering=False, debug=False,
                   num_devices=N_CORES)
    xT = nc.dram_tensor("xT", [CA, N], bf, kind="ExternalInput").ap()
    wqk = nc.dram_tensor("wqk", [CA, 2 * CL], bf, kind="ExternalInput").ap()
    wv = nc.dram_tensor("wv", [CA, CL], bf, kind="ExternalInput").ap()
    wp = nc.dram_tensor("wp", [CL, C], bf, kind="ExternalInput").ap()
    wb = nc.dram_tensor("wb", [128, 8], bf, kind="ExternalInput").ap()
    vb = nc.dram_tensor("vb", [1, CL], bf, kind="ExternalInput").ap()
    outT = nc.dram_tensor("outT", [C, N], f32, kind="ExternalOutput").ap()

    xT_r = xT.rearrange("(k p) n -> k p n", p=128)
    wqk_r = wqk.rearrange("(k p) n -> k p n", p=128)
    wv_r = wv.rearrange("(k p) n -> k p n", p=128)
    wp_r = wp.rearrange("(k p) n -> k p n", p=128)

    with tile.TileContext(nc) as tc, ExitStack() as ctx:
        const = ctx.enter_context(tc.tile_pool(name="const", bufs=1))
        x_bufs = [const.tile([128, KC, N], bf, name="x%d" % i)
                  for i in range(2 if loop_n > 1 else 1)]
        wqk_sb = const.tile([128, KC, 2 * CL], bf)
        wv_sb = const.tile([128, KC, CL], bf)
        wp_sb = const.tile([128, 4, C], bf)
        qk_sb = const.tile([128, 8, N], bf)        # [feat%128, feat_tile, tok]
        v_sb = const.tile([128, NKC, HL * VW], bf)  # v + ones col + pad per head
        o_sb = const.tile([128, 4, N], bf)         # o_norm^T [cloc%128, chunk, tok]

        # PSUM: s 2x2 banks + av 1x2 banks + mm 2x1 banks = 8 banks exactly
        sps = ctx.enter_context(tc.tile_pool(name="sps", bufs=2, space="PSUM"))
        avps = ctx.enter_context(tc.tile_pool(name="avps", bufs=1, space="PSUM"))
        mmps = ctx.enter_context(tc.tile_pool(name="mmps", bufs=2, space="PSUM"))
        pp = ctx.enter_context(tc.tile_pool(name="pp", bufs=6))
        normp = ctx.enter_context(tc.tile_pool(name="normp", bufs=1))
        onp = ctx.enter_context(tc.tile_pool(name="onp", bufs=2))
        osp = ctx.enter_context(tc.tile_pool(name="osp", bufs=4))

        # weights resident across iterations
        for k in range(KC):
            nc.scalar.dma_start(wqk_sb[:, k, :], wqk_r[k])
            nc.scalar.dma_start(wv_sb[:, k, :], wv_r[k])
        for k in range(4):
            nc.scalar.dma_start(wp_sb[:, k, :], wp_r[k])
        wb_sb = const.tile([128, 8], bf)
        vb_row = const.tile([1, CL], bf)
        vb_sb = const.tile([128, CL], bf)   # v bias replicated across tokens
        nc.scalar.dma_start(wb_sb[:], wb)
        nc.scalar.dma_start(vb_row[:], vb)
        nc.gpsimd.partition_broadcast(vb_sb[:], vb_row[:], channels=128)
        v_hl = v_sb.rearrange("p t (h e) -> p t h e", e=VW)
        nc.vector.memset(v_hl[:, :, :, 64:65], 1.0)
        nc.vector.memset(v_hl[:, :, :, 65:66], 0.0)

        def _qk_pair(x_sb, f, parts=(0, 1), tb_outer=False):
            # feature tiles f (q) and 4+f (k); weight chunk stays in flight
            # only per-MM (LDW hides under the N=512 stream). tb_outer
            # consumes x token-slices as the DMA delivers them (startup).
            order = ([(tb, sel) for tb in range(TB) for sel in parts]
                     if tb_outer else
                     [(tb, sel) for sel in parts for tb in range(TB)])
            for tb, sel in order:
                ft = f if sel == 0 else 4 + f
                ps = mmps.tile([128, 512], f32, tag="mm")
                for k in range(KC):
                    nc.tensor.matmul(
                        ps[:],
                        wqk_sb[:, k, ft * 128:(ft + 1) * 128],
                        x_sb[:, k, tb * 512:(tb + 1) * 512],
                        start=(k == 0), stop=(k == KC - 1),
                    )
                nc.vector.tensor_add(
                    qk_sb[:, ft, tb * 512:(tb + 1) * 512], ps[:],
                    wb_sb[:, ft:ft + 1].to_broadcast([128, 512]))

        def _v_tile(x_sb, tt):
            ps = mmps.tile([128, 512], f32, tag="mm")
            for k in range(KC):
                nc.tensor.matmul(
                    ps[:],
                    x_sb[:, k, tt * 128:(tt + 1) * 128],
                    wv_sb[:, k, :],
                    start=(k == 0), stop=(k == KC - 1),
                )
            v_out = v_sb[:, tt, :].rearrange("p (h e) -> p h e", e=VW)[:, :, 0:64]
            v_in = ps[:].rearrange("p (h e) -> p h e", e=64)
            nc.vector.tensor_add(v_out, v_in,
                                 vb_sb.rearrange("p (h e) -> p h e", e=64))

        def _attn_blk(h, blk, fill=None):
            hrow = 64 * (h % 2)
            f = h // 2
            nq0 = blk * 1024
            q_ap = qk_sb[hrow:hrow + 64, f, :]
            k_ap = qk_sb[hrow:hrow + 64, 4 + f, :]
            av = avps.tile([VW, 1024], f32, tag="av")
            for ck in range(NKC):
                s = sps.tile([128, 1024], f32, tag="s")
                for q in range(2):
                    nc.tensor.matmul(
                        s[:, q * 512:(q + 1) * 512],
                        k_ap[:, ck * 128:(ck + 1) * 128],
                        q_ap[:, nq0 + q * 512:nq0 + (q + 1) * 512],
                        start=True, stop=True)
                p = pp.tile([128, 1024], bf, tag="p")
                nc.scalar.activation(p[:], s[:], AF.Exp)
                if fill is not None:
                    fill(ck)
                for q in range(2):
                    nc.tensor.matmul(
                        av[:, q * 512:(q + 1) * 512],
                        v_sb[:, ck, h * VW:(h + 1) * VW],
                        p[:, q * 512:(q + 1) * 512],
                        start=(ck == 0), stop=(ck == NKC - 1))
            # normalize: o = av[0:64] * (1/Z), Z = av row 64.
            # Evacuation copies release the PSUM accumulator ~2.3us after
            # the last AV matmul; 1/Z via reciprocal_approx_fast (51 ULP,
            # fp32 SBUF->SBUF) instead of the ~8 cyc/elem iterative-divide
            # reciprocal(), which on a single-lane [1,1024] AP costs ~8.5us.
            oc = normp.tile([64, 1024], bf, tag="oc")
            with nc.allow_low_precision(reason="o in bf16 (o_sb is bf16)"):
                nc.vector.tensor_copy(oc[:], av[0:64, :])
            zc = normp.tile([1, 1024], f32, tag="zc")
            nc.vector.tensor_copy(zc[:], av[64:65, :])
            r32 = normp.tile([1, 1024], f32, tag="r32")
            nc.vector.reciprocal_approx_fast(r32[:], zc[:])
            r = normp.tile([1, 1024], bf, tag="r")
            with nc.allow_low_precision(reason="1/Z in bf16; validated e2e"):
                nc.vector.tensor_copy(r[:], r32[:])
            bc = normp.tile([64, 1024], bf, tag="bc")
            nc.gpsimd.partition_broadcast(bc[:], r[:], channels=64)
            if h % 2 == 0:
                nc.vector.tensor_mul(
                    o_sb[0:64, f, nq0:nq0 + 1024], oc[:], bc[:])
            else:
                on = onp.tile([64, 1024], bf, tag="on")
                nc.vector.tensor_mul(on[:], oc[:], bc[:])
                nc.scalar.dma_start(o_sb[64:128, f, nq0:nq0 + 1024], on[:])

        def _proj_blk(blk):
            # wp chunk stays stationary for the 2 matmuls of the tb pair
            for ct in range(8):
                pss = [mmps.tile([128, 512], f32, tag="mm", name="pj%d" % i)
                       for i in range(2)]
                for k in range(4):
                    for i in range(2):
                        tb = blk * 2 + i
                        nc.tensor.matmul(
                            pss[i][:],
                            wp_sb[:, k, ct * 128:(ct + 1) * 128],
                            o_sb[:, k, tb * 512:(tb + 1) * 512],
                            start=(k == 0), stop=(k == 3),
                        )
                for i in range(2):
                    tb = blk * 2 + i
                    ost = osp.tile([128, 512], f32, tag="o")
                    nc.vector.tensor_copy(ost[:], pss[i][:])
                    nc.scalar.dma_start(
                        outT[ct * 128:(ct + 1) * 128,
                             tb * 512:(tb + 1) * 512],
                        ost[:])

        def _dma_x(x_sb):
            # x streamed token-block-major on the sync queue (weights use the
            # scalar queue) so the first qkv matmuls start after ~1/4 of it
            for tb in range(TB):
                for k in range(KC):
                    nc.sync.dma_start(
                        x_sb[:, k, tb * 512:(tb + 1) * 512],
                        xT_r[k][:, tb * 512:(tb + 1) * 512])

        def _body(x_sb):
            _dma_x(x_sb)
            _qk_pair(x_sb, 0, tb_outer=True)
            for tt in range(4):
                _v_tile(x_sb, tt)
            # v tiles 4..15 interleave into head 0's first block (emitted
            # ahead of the AV matmuls that consume them)
            _attn_blk(0, 0,
                      fill=lambda ck: _v_tile(x_sb, ck + 4) if ck < 12 else None)
            _attn_blk(0, 1)
            _attn_blk(1, 0)
            _qk_pair(x_sb, 1, parts=(0,))
            _attn_blk(1, 1)
            _qk_pair(x_sb, 1, parts=(1,))
            _attn_blk(2, 0)
            _attn_blk(2, 1)
            _qk_pair(x_sb, 2, parts=(0,))
            _attn_blk(3, 0)
            _qk_pair(x_sb, 2, parts=(1,))
            _attn_blk(3, 1)
            _attn_blk(4, 0)
            _qk_pair(x_sb, 3, parts=(0,))
            _attn_blk(4, 1)
            _qk_pair(x_sb, 3, parts=(1,))
            _attn_blk(5, 0)
            _attn_blk(5, 1)
            _attn_blk(6, 0)
            _attn_blk(7, 0)
            _proj_blk(0)
            _attn_blk(7, 1)
            _attn_blk(6, 1)
            _proj_blk(1)

        if loop_n > 1:
            assert loop_n % 4 == 0, "loop_n must be a multiple of 4"
            with tc.For_i(0, loop_n // 4, 1,
                          hint_engines=(mybir.EngineType.PE,
                                        mybir.EngineType.Activation)):
                # 4 bodies alternating 2 x buffers: each body's x DMA
                # sits early in the sync queue and streams during the
                # previous body (its WAR deps clear 2 bodies ahead)
                for rep in range(4):
                    _body(x_bufs[rep % 2])
        else:
            _body(x_bufs[0])

    nc.compile()
    return nc


def _prep_core_inputs(x, w_qkv, b_qkv, w_proj, core):
    b, g = core // 2, core % 2
    scale = np.float32(D) ** -0.5

    xT = np.ascontiguousarray(x[b].T).astype(BF)

    q_w = w_qkv[g * CL:(g + 1) * CL] * scale
    k_w = w_qkv[C + g * CL:C + (g + 1) * CL]
    v_w = w_qkv[2 * C + g * CL:2 * C + (g + 1) * CL]
    q_b = b_qkv[g * CL:(g + 1) * CL] * scale
    k_b = b_qkv[C + g * CL:C + (g + 1) * CL]
    v_b = b_qkv[2 * C + g * CL:2 * C + (g + 1) * CL]

    wqk_c = np.empty((CA, 2 * CL), dtype=BF)
    wqk_c[:, :CL] = q_w.T.astype(BF)
    wqk_c[:, CL:] = k_w.T.astype(BF)

    # per-feature-tile bias columns: ft 0-3 = q tiles, 4-7 = k tiles
    wb = np.empty((128, 8), dtype=BF)
    for f in range(4):
        wb[:, f] = q_b[f * 128:(f + 1) * 128].astype(BF)
        wb[:, 4 + f] = k_b[f * 128:(f + 1) * 128].astype(BF)

    wpT = np.ascontiguousarray(w_proj[:, g * CL:(g + 1) * CL].T).astype(BF)

    return {"xT": xT, "wqk": wqk_c, "wv": v_w.T.astype(BF),
            "wp": wpT, "wb": wb, "vb": v_b.astype(BF).reshape(1, CL)}


def kernel(x, w_qkv, b_qkv, w_proj, b_proj):
    from concourse.bass_utils import run_bass_kernel_spmd

    x = np.asarray(x, dtype=np.float32)
    w_qkv = np.asarray(w_qkv, dtype=np.float32)
    b_qkv = np.asarray(b_qkv, dtype=np.float32)
    w_proj = np.asarray(w_proj, dtype=np.float32)
    b_proj = np.asarray(b_proj, dtype=np.float32)

    if "nc" not in _CACHE:
        _CACHE["nc"] = _build()
    nc = _CACHE["nc"]

    in_maps = [_prep_core_inputs(x, w_qkv, b_qkv, w_proj, c)
               for c in range(N_CORES)]
    res = run_bass_kernel_spmd(nc, in_maps, core_ids=list(range(N_CORES)))
    _CACHE["last_results"] = res

    out = np.empty((B, N, C), dtype=np.float32)
    for b in range(B):
        acc = res.results[2 * b]["outT"] + res.results[2 * b + 1]["outT"]
        out[b] = acc.T + b_proj[None, :]
    return out


def _make_runner(nc, in_maps):
    """jit-wrap a compiled module for repeated dispatch on the 8 cores."""
    import jax
    from concourse import bass2jax, mybir
    from jax.sharding import Mesh, PartitionSpec, NamedSharding

    bass2jax.install_neuronx_cc_hook()
    part_name = (nc.partition_id_tensor.name
                 if nc.partition_id_tensor is not None else None)
    in_names, out_names, out_avals, zero_outs = [], [], [], []
    for alloc in nc.m.functions[0].allocations:
        if not isinstance(alloc, bass2jax.mybir.MemoryLocationSet):
            continue
        name = alloc.memorylocations[0].name
        if alloc.kind == "ExternalInput":
            if name != part_name:
                in_names.append(name)
        elif alloc.kind == "ExternalOutput":
            out_names.append(name)
            shape = tuple(alloc.tensor_shape)
            dtype = mybir.dt.np(alloc.dtype)
            out_avals.append(jax.core.ShapedArray(shape, dtype))
            zero_outs.append(np.zeros(shape, dtype))
    n_params = len(in_names)
    n_outs = len(out_avals)
    all_names = in_names + out_names
    if part_name is not None:
        all_names = all_names + [part_name]

    def _bodyfn(*args):
        operands = list(args)
        if part_name is not None:
            operands.append(bass2jax.partition_id_tensor())
        outs = bass2jax._bass_exec_p.bind(
            *operands,
            out_avals=tuple(out_avals),
            in_names=tuple(all_names),
            out_names=tuple(out_names),
            lowering_input_output_aliases=(),
            sim_require_finite=True,
            sim_require_nnan=True,
            nc=nc,
        )
        return tuple(outs)

    devices = jax.devices()[:N_CORES]
    mesh = Mesh(np.asarray(devices), ("core",))
    spec = PartitionSpec("core")
    sharded = jax.jit(
        bass2jax.shard_map(_bodyfn, mesh=mesh,
                           in_specs=(spec,) * (n_params + n_outs),
                           out_specs=(spec,) * n_outs, check_rep=False),
        keep_unused=True)

    concat_in = [
        np.concatenate([np.asarray(in_maps[c][name]) for c in range(N_CORES)],
                       axis=0)
        for name in in_names
    ]
    sh = NamedSharding(mesh, spec)
    dev_in = [jax.device_put(a, sh) for a in concat_in]
    dev_zeros = [jax.device_put(
        np.zeros((N_CORES * z.shape[0], *z.shape[1:]), z.dtype), sh)
        for z in zero_outs]

    def run():
        outs = sharded(*dev_in, *dev_zeros)
        jax.block_until_ready(outs)
        return outs

    return run


def benchmark(x, w_qkv, b_qkv, w_proj, b_proj, iters=20, reps=4):
    """HW execution time of one kernel body.

    Measured as the slope between two hardware For-loop NEFFs running the
    identical body `iters` and `iters+100` times on-device: per-body ns =
    (T_long - T_short) / 100.  The differencing cancels everything that is
    not device execution of the body (host dispatch RPC, NEFF entry/exit,
    input/output transfer), which otherwise dominates wall-clock here
    (~60ms/dispatch through the axon tunnel vs ~0.5ms of device time).
    Both points are best-of-`reps` dispatches.

    Test-harness helper only (not used by kernel()).
    """
    import time

    x = np.asarray(x, dtype=np.float32)
    in_maps = [_prep_core_inputs(x, np.asarray(w_qkv, np.float32),
                                 np.asarray(b_qkv, np.float32),
                                 np.asarray(w_proj, np.float32), c)
               for c in range(N_CORES)]

    n1 = max(4, ((iters + 3) // 4) * 4)   # loop NEFF needs a multiple of 4
    n2 = n1 + 100
    runners = {}
    for loop_n in (n1, n2):
        key = ("loop", loop_n)
        if key not in _CACHE:
            _CACHE[key] = _build(loop_n=loop_n)
        runners[loop_n] = _make_runner(_CACHE[key], in_maps)
        runners[loop_n]()  # warmup: compiles + loads NEFF
    # Alternate the two points so transient host/device slowness (e.g.
    # post-reset warmup) cannot land entirely on one point and skew the
    # slope; best-of per point over the interleaved reps.
    best = {n1: None, n2: None}
    for _ in range(max(reps, 5)):
        for loop_n in (n1, n2):
            t0 = time.perf_counter()
            runners[loop_n]()
            t1 = time.perf_counter()
            dt = t1 - t0
            if best[loop_n] is None or dt < best[loop_n]:
                best[loop_n] = dt
    return (best[n2] - best[n1]) / (n2 - n1) * 1e9
